# revision 1
# baseline (speedup 1.0000x reference)
"""Trainium2 Bass kernel v2 for nn_Capsule_2731599200537 (capsule routing).

Math (per core, i-sharded NIS=256):
    votes[b,i,ja] = sum_k x[b,i,k] W[i,k,ja]
    r1: preact = (sum_i votes)/33        (dense (ik) matmul, no votes needed)
    r>1: logits_r = votes . actsum_{r-1} (actsum = act1+...+act_{r-1}; linearity)
         route = leaky-softmax(logits); preact = route . votes via xr=x*route
    AllReduce preact partials each round; act = squash(preact + b).

"""
from contextlib import nullcontext

import numpy as np

import concourse.bacc as bacc
import concourse.mybir as mybir
from concourse import tile
from concourse.bass_utils import run_bass_kernel_spmd
from concourse.masks import make_identity

B = 64
NI = 2048
KA = 8
NO = 32
AT = 16
O = NO * AT
R = 3
NCORES = 8
NIS = NI // NCORES
NH = NIS // 128
NT = NIS // 16

F32 = mybir.dt.float32
F16 = mybir.dt.float16
F8 = mybir.dt.float8e4
AF = mybir.ActivationFunctionType
AX = mybir.AxisListType
MUL = mybir.AluOpType.mult


class Bal:
    """Greedy per-engine load balancer for elementwise/copy ops."""

    def __init__(self, nc, ante_v=0.0, no_pool=True):
        self.nc = nc
        self.no_pool = no_pool
        # ante_v: reserved future DVE-only work (xr/reduce in rounds) so the
        # phase-1 greedy doesn't overload DVE with evictions.
        self.load = {"v": ante_v, "s": 0.0, "p": 0.0}

    def copy(self, dst, src, elems, f16=False, engines="vsp", psum=False):
        if self.no_pool:
            engines = engines.replace("p", "") or "v"
        if psum:
            engines = engines.replace("p", "")  # GPSIMD cannot access PSUM
        costs = {}
        if "v" in engines:
            costs["v"] = elems * (0.52 if f16 else 1.04) + 110
        if "s" in engines:
            costs["s"] = elems * 0.833 + 220
        if "p" in engines:
            costs["p"] = elems * 1.39 + 130
        e = min(costs, key=lambda k: self.load[k] + costs[k])
        self.load[e] += costs[e]
        if e == "v":
            return self.nc.vector.tensor_copy(dst, src)
        if e == "s":
            return self.nc.scalar.copy(dst, src)
        return self.nc.gpsimd.tensor_copy(dst, src)

    def tt(self, out, in1, in2, elems, fast=False, engines="vp",
           op=MUL):
        costs = {}
        if "v" in engines:
            costs["v"] = elems * (0.52 if fast else 1.04) + 110
        if "p" in engines and not fast:
            costs["p"] = elems * 2.0 + 150
        e = min(costs, key=lambda k: self.load[k] + costs[k])
        self.load[e] += costs[e]
        eng = self.nc.vector if e == "v" else self.nc.gpsimd
        return eng.tensor_tensor(out, in1, in2, op=op)

    def note(self, eng, ns):
        self.load[eng] += ns


def build(n_cores: int = NCORES, use_collective: bool = True,
          cc_rounds=(1, 2, 3), loop_n: int | None = None, parts: int = 4,
          cc_chunks: int = 1, overlap_cc1: bool = True, no_pool: bool = True):
    nc = bacc.Bacc(None, target_bir_lowering=False, debug=False,
                   num_devices=n_cores)
    x_d = nc.dram_tensor("x", [B, NIS, KA], F32, kind="ExternalInput")
    w_d = nc.dram_tensor("w", [NIS, KA, O], F32, kind="ExternalInput")
    b_d = nc.dram_tensor("b", [NO, AT], F32, kind="ExternalInput")
    y_d = nc.dram_tensor("y", [B, NO, AT], F32, kind="ExternalOutput")

    wik = w_d.rearrange("i k o -> (i k) o")    # [2048, 512]
    wko = w_d.rearrange("i k o -> i (k o)")    # [256, 4096]

    bal_holder = {}

    with tile.TileContext(nc) as tc:
        with (
            tc.tile_pool(name="big", bufs=1) as big,
            tc.tile_pool(name="cst", bufs=1) as cst,
            tc.tile_pool(name="psA", bufs=6, space="PSUM") as psA,
            tc.tile_pool(name="psT", bufs=1, space="PSUM") as psT,
            tc.tile_pool(name="psB", bufs=1, space="PSUM") as psB,
            tc.tile_pool(name="dram", bufs=2, space="DRAM") as dram,
        ):
            bal = Bal(nc, ante_v=22000.0, no_pool=no_pool)
            bal_holder["bal"] = bal
            # ---- persistent SBUF ----
            # votes fp8 [p=(ja)_c, (b, i)]
            v8 = [big.tile([128, B * NIS], F16, tag=f"v8_{c}", name=f"v8_{c}")
                  for c in range(4)]
            v8v = [t.rearrange("p (b i) -> p b i", b=B) for t in v8]
            w16 = big.tile([128, NT * O], F16, tag="w16", name="w16")
            w16v = w16.rearrange("p (t o) -> p t o", t=NT)
            w2 = [big.tile([128, KA * O], F16, tag=f"w2_{h}", name=f"w2_{h}")
                  for h in range(NH)]
            w2v = [t.rearrange("p (k o) -> p k o", k=KA) for t in w2]
            xt16 = big.tile([128, NT * B], F16, tag="xt16", name="xt16")
            xt16v = xt16.rearrange("p (t b) -> p t b", t=NT)
            xt2k = [big.tile([128, KA * B], F16, tag=f"xt2k{h}", name=f"x2k{h}")
                    for h in range(NH)]
            xt2kv = [t.rearrange("p (k b) -> p k b", k=KA) for t in xt2k]

            ident = cst.tile([64, 64], F16)
            make_identity(nc, ident[:])
            bias_bc = cst.tile([64, O], F32)
            brow = cst.tile([1, O], F32)
            nc.sync.dma_start(brow[:], b_d.rearrange("j a -> (j a)").unsqueeze(0))
            nc.gpsimd.partition_broadcast(bias_bc[:], brow[:])

            # 4-slot PSUM transpose buffer: transposes don't serialize
            ptq = psT.tile([128, 256], F16, tag="ptT", name="ptq")
            slot = [0]

            def transpose_evict(src_ap, dst_ap):
                s = slot[0] % 4
                slot[0] += 1
                nc.tensor.transpose(ptq[:, 64 * s:64 * (s + 1)], src_ap,
                                    ident[:])
                bal.copy(dst_ap, ptq[:, 64 * s:64 * (s + 1)], 64, f16=True, psum=True)

            _loop = tc.For_i(0, loop_n, 1) if loop_n else nullcontext()
            _loop.__enter__()
            # =========== phase 1: loads, converts, r1 preact, votes ========
            with (
                tc.tile_pool(name="p1", bufs=1) as p1,
                tc.tile_pool(name="stg", bufs=4) as stg,
                tc.tile_pool(name="xbp", bufs=4) as xbp,
            ):
                # x -> fp16 [b, (i k)]  (one DMA, two converts)
                xnat16 = p1.tile([64, NIS * KA], F16)
                if parts >= 1:
                    stx = p1.tile([64, 2048], F32, tag="stagex")
                    nc.sync.dma_start(stx[:],
                                      x_d.rearrange("b i k -> b (i k)"))
                    for c in range(2):
                        bal.copy(xnat16[:, 1024 * c:1024 * (c + 1)],
                                 stx[:, 1024 * c:1024 * (c + 1)], 1024)

                # W -> fp16 [(i k), ja]  (8 DMAs of 2 t-chunks each)
                wik2 = wik.rearrange("(u t2 p) o -> u p t2 o", t2=2, p=128)
                for u in range(8 if parts >= 1 else 0):
                    st = stg.tile([128, 1024], F32, tag="stage")
                    nc.sync.dma_start(
                        st.rearrange("p (t2 o) -> p t2 o", t2=2), wik2[u])
                    for v in range(2):
                        bal.copy(w16v[:, 2 * u + v, :],
                                 st[:, 512 * v:512 * (v + 1)], 512)

                # W -> fp16 [i, (k ja)]  (4 DMAs per half)
                for h in range(NH if parts >= 1 else 0):
                    for c in range(4):
                        st = stg.tile([128, 1024], F32, tag="stage")
                        nc.sync.dma_start(
                            st[:], wko[128 * h:128 * (h + 1),
                                       1024 * c:1024 * (c + 1)])
                        for v in range(2):
                            bal.copy(w2[h][:, 1024 * c + 512 * v:
                                           1024 * c + 512 * (v + 1)],
                                     st[:, 512 * v:512 * (v + 1)], 512)

                # xT16 [(i k), b]
                for t in range(NT if parts >= 1 else 0):
                    transpose_evict(xnat16[:, 128 * t:128 * (t + 1)],
                                    xt16v[:, t, :])

                # xt2k [i, (k, b)]
                xnk = xnat16.rearrange("b (i k) -> b i k", k=KA)
                for h in range(NH if parts >= 1 else 0):
                    for k in range(KA):
                        transpose_evict(xnk[:, 128 * h:128 * (h + 1), k],
                                        xt2kv[h][:, k, :])

                # block-diag selection mask [128, 16]
                mask = p1.tile([128, 16], F16)
                nc.gpsimd.memset(mask[:], 1.0)
                nc.gpsimd.affine_select(
                    out=mask[:], in_=mask[:],
                    compare_op=mybir.AluOpType.is_ge, fill=0.0,
                    base=0, pattern=[[-8, 16]], channel_multiplier=1)
                nc.gpsimd.affine_select(
                    out=mask[:], in_=mask[:],
                    compare_op=mybir.AluOpType.is_ge, fill=0.0,
                    base=7, pattern=[[8, 16]], channel_multiplier=-1)

                # ---- round-1 preact (before production; CC1 overlaps) ----
                psb1 = psB.tile([64, O], F32, tag="psb")
                for t in range(NT if parts >= 2 else 0):
                    nc.tensor.matmul(psb1[:], xt16v[:, t, :], w16v[:, t, :],
                                     start=(t == 0), stop=(t == NT - 1))

                # ---- votes production ----
                for t in range(NT if parts >= 2 else 0):
                    xb = xbp.tile([128, B * 16], F16, tag="xb")
                    xbv = xb.rearrange("p (b i) -> p b i", b=B)
                    bal.tt(xbv,
                           mask.unsqueeze(1).broadcast_to([128, B, 16]),
                           xt16v[:, t, :].unsqueeze(2).broadcast_to(
                               [128, B, 16]),
                           1024)
                    for c2 in range(4):
                        for h in range(2):
                            bank = psA.tile([128, 512], F32, tag="bank")
                            nc.tensor.matmul(
                                bank[:],
                                w16v[:, t, 128 * c2:128 * (c2 + 1)],
                                xb[:, 512 * h:512 * (h + 1)],
                                start=True, stop=True)
                            bkv = bank.rearrange("p (b i) -> p b i", b=32)
                            bal.copy(
                                v8v[c2][:, 32 * h:32 * (h + 1),
                                        16 * t:16 * (t + 1)],
                                bkv, 512, psum=True)

            # =========== routing ===========
            with (
                tc.tile_pool(name="rt", bufs=1) as rt,
                tc.tile_pool(name="xrp", bufs=2) as xrp,
            ):
                routef = [rt.tile([128, NO * B], F16, tag=f"rf{h}", name=f"rf{h}")
                          for h in range(NH)]          # [i, (j, b)]
                rfv = [t.rearrange("p (j b) -> p j b", j=NO) for t in routef]
                actT = [rt.tile([128, 64], F16, tag=f"actT{c}", name=f"acT{c}")
                        for c in range(4)]
                av = [rt.tile([128, B * 8], F16, tag=f"ab{c}", name=f"ab{c}")
                      for c in range(4)]               # f16 actblk
                avv = [t.rearrange("p (b j) -> p b j", b=B) for t in av]
                maskJ = rt.tile([128, 8], F16, tag="maskJ")
                nc.gpsimd.memset(maskJ[:], 1.0)
                nc.gpsimd.affine_select(
                    out=maskJ[:], in_=maskJ[:],
                    compare_op=mybir.AluOpType.is_ge, fill=0.0,
                    base=0, pattern=[[-16, 8]], channel_multiplier=1)
                nc.gpsimd.affine_select(
                    out=maskJ[:], in_=maskJ[:],
                    compare_op=mybir.AluOpType.is_ge, fill=0.0,
                    base=15, pattern=[[16, 8]], channel_multiplier=-1)

                xd = [rt.tile([128, KA * B], F16, tag=f"xd{h}", name=f"xd{h}")
                      for h in range(NH)]
                xdv = [t.rearrange("p (k b) -> p k b", k=KA) for t in xd]
                pre_part = rt.tile([64, O], F32, tag="pre_part")
                pre_sum = rt.tile([64, O], F32, tag="pre_sum")
                sq = rt.tile([64, O], F32, tag="sq")
                nsq = rt.tile([64, NO], F32, tag="nsq")
                norm = rt.tile([64, NO], F32, tag="norm")
                d1 = rt.tile([64, NO], F32, tag="d1")
                rd = rt.tile([64, NO], F32, tag="rd")
                fs = rt.tile([64, NO], F32, tag="fs")
                acts = rt.tile([64, O], F32, tag="acts")
                act16 = rt.tile([64, O], F16, tag="act16")
                asum32 = rt.tile([64, O], F32, tag="asum32")
                asum16 = rt.tile([64, O], F16, tag="asum16")
                denom = [rt.tile([128, B], F32, tag=f"den{h}", name=f"den{h}")
                         for h in range(NH)]
                recip = [rt.tile([128, B], F32, tag=f"rec{h}", name=f"rec{h}")
                         for h in range(NH)]
                recip16 = [rt.tile([128, B], F16, tag=f"re6{h}", name=f"re6{h}")
                           for h in range(NH)]

                def cc_reduce(r):
                    """pre_part -> pre_sum (+bias), chunked AllReduce."""
                    nch = cc_chunks if r > 1 else 1
                    W_ = O // nch
                    for ch in range(nch):
                        sl = slice(W_ * ch, W_ * (ch + 1))
                        if use_collective and r in cc_rounds:
                            cc_in = dram.tile([64, W_], F32, tag="ccin")
                            cc_out = dram.tile([64, W_], F32, tag="ccout")
                            nc.sync.dma_start(cc_in[:], pre_part[:, sl])
                            nc.gpsimd.collective_compute(
                                "AllReduce", mybir.AluOpType.add,
                                replica_groups=[list(range(n_cores))],
                                ins=[cc_in.opt()], outs=[cc_out.opt()])
                            nc.sync.dma_start(pre_sum[:, sl], cc_out[:])
                        else:
                            bal.copy(pre_sum[:, sl], pre_part[:, sl], W_, engines="sp")
                        nc.vector.tensor_add(pre_sum[:, sl], pre_sum[:, sl],
                                             bias_bc[:, sl])
                        bal.note("v", W_ * 1.04 + 110)

                def squash(r):
                    """pre_sum -> acts -> act16/asum; per cc-chunk slices."""
                    nch = cc_chunks if r > 1 else 1
                    JW = NO // nch
                    for ch in range(nch):
                        sl = slice(AT * JW * ch, AT * JW * (ch + 1))
                        jl = slice(JW * ch, JW * (ch + 1))
                        nc.vector.tensor_mul(sq[:, sl], pre_sum[:, sl],
                                             pre_sum[:, sl])
                        nc.vector.reduce_sum(
                            nsq[:, jl],
                            sq[:, sl].rearrange("p (j a) -> p j a", a=AT),
                            axis=AX.X)
                        nc.scalar.activation(norm[:, jl], nsq[:, jl], AF.Sqrt)
                        nc.vector.tensor_scalar_add(d1[:, jl], nsq[:, jl], 1.0)
                        nc.vector.reciprocal(rd[:, jl], d1[:, jl])
                        nc.vector.tensor_mul(fs[:, jl], norm[:, jl], rd[:, jl])
                        nc.vector.tensor_mul(
                            acts[:, sl].rearrange("p (j a) -> p j a", a=AT),
                            pre_sum[:, sl].rearrange("p (j a) -> p j a", a=AT),
                            fs[:, jl].unsqueeze(2).broadcast_to([64, JW, AT]))
                        bal.note("v", 512 * 3 / nch * 1.04 + 500)
                        if r < R:
                            bal.copy(act16[:, sl], acts[:, sl], O // nch, engines="sp")
                            if r == 1:
                                bal.copy(asum32[:, sl], acts[:, sl], O // nch, engines="sp")
                            else:
                                nc.vector.tensor_add(asum32[:, sl],
                                                     asum32[:, sl],
                                                     acts[:, sl])
                                bal.note("v", O / nch * 1.04 + 110)
                                bal.copy(asum16[:, sl], asum32[:, sl], O // nch, engines="sp")
                        else:
                            nc.sync.dma_start(
                                y_d.rearrange("b j a -> b (j a)")[:, sl],
                                acts[:, sl])

                # ---- round 1 ----
                if parts >= 2:
                    nc.scalar.mul(pre_part[:], psb1[:], 1.0 / 33.0)
                    bal.note("s", 512 * 0.833 + 400)
                    cc_reduce(1)
                    squash(1)

                # ---- rounds 2..R ----
                for r in range(2, (R + 1) if parts >= 3 else 2):
                    actsrc = act16 if r == 2 else asum16
                    # actT + actblk
                    for c in range(4):
                        transpose_evict(actsrc[:, 128 * c:128 * (c + 1)],
                                        actT[c][:])
                        bal.tt(avv[c],
                               maskJ.unsqueeze(1).broadcast_to([128, B, 8]),
                               actT[c].unsqueeze(2).broadcast_to([128, B, 8]),
                               512)

                    # dlogit -> PSUM banks; exp straight out of PSUM
                    for h in range(NH):
                        for bg in range(4):
                            bank = psA.tile([128, 512], F32, tag="bank")
                            for b16 in range(16):
                                b = 16 * bg + b16
                                for c in range(4):
                                    off = 32 * b16 + 8 * c
                                    nc.tensor.matmul(
                                        bank[:, off:off + 8],
                                        v8v[c][:, b, 128 * h:128 * (h + 1)],
                                        avv[c][:, b, :],
                                        start=(c == 0), stop=(c == 3))
                            # exp: bank [i,(b16,j32)] -> rfv [i,(j,b16 slice)]
                            bkv = bank.rearrange("p (b j) -> p b j", b=16)
                            nc.scalar.activation(
                                rfv[h][:, :, 16 * bg:16 * (bg + 1)]
                                .transpose([0, 2, 1]),
                                bkv, AF.Exp)
                            bal.note("s", 512 * 0.833 + 400)

                    # per-h: softmax denom -> xd -> xr -> preact matmuls.
                    # h0's xr/preact runs while h1's exps still in flight;
                    # each psb j-slice accumulates h0:k0..7 then h1:k0..7.
                    psb = psB.tile([64, O], F32, tag="psb")

                    def softmax_h(h):
                        nc.vector.reduce_sum(
                            denom[h][:], rfv[h].transpose([0, 2, 1]),
                            axis=AX.X)
                        bal.note("v", 2048 * 1.04 + 300)
                        nc.vector.tensor_scalar_add(denom[h][:], denom[h][:],
                                                    1.0)
                        nc.vector.reciprocal(recip[h][:], denom[h][:])
                        nc.vector.tensor_copy(recip16[h][:], recip[h][:])
                        bal.note("v", 64 * 3 * 1.04 + 330)
                        nc.vector.tensor_mul(
                            xdv[h],
                            xt2kv[h],
                            recip16[h].unsqueeze(1).broadcast_to(
                                [128, KA, B]))
                        bal.note("v", 512 * 0.52 + 110)

                    softmax_h(0)
                    softmax_h(1)
                    for jg in range(8):
                        xrt = [xrp.tile([128, 4 * KA * B], F16, tag="xr",
                                        name=f"xrt{h}")
                               for h in range(NH)]
                        for h in range(NH):
                            xv = xrt[h].rearrange("p (j k b) -> p j k b",
                                                  j=4, k=KA)
                            bal.tt(xv,
                                   xdv[h].unsqueeze(1).broadcast_to(
                                       [128, 4, KA, B]),
                                   rfv[h][:, 4 * jg:4 * (jg + 1), :]
                                   .unsqueeze(2).broadcast_to([128, 4, KA, B]),
                                   2048, fast=True)
                        for jj in range(4):
                            j = 4 * jg + jj
                            for h in range(NH):
                                for k in range(KA):
                                    nc.tensor.matmul(
                                        psb[:, 16 * j:16 * (j + 1)],
                                        xrt[h][:, 512 * jj + 64 * k:
                                               512 * jj + 64 * (k + 1)],
                                        w2v[h][:, k, 16 * j:16 * (j + 1)],
                                        start=(h == 0 and k == 0),
                                        stop=(h == NH - 1 and k == KA - 1))
                    bal.copy(pre_part[:], psb[:], 512, engines="s")
                    cc_reduce(r)
                    squash(r)
            _loop.__exit__(None, None, None)

    nc.compile()
    return nc


_NC_CACHE = {}


def _get_nc(n_cores=NCORES, use_collective=True):
    key = (n_cores, use_collective)
    if key not in _NC_CACHE:
        _NC_CACHE[key] = build(n_cores, use_collective)
    return _NC_CACHE[key]


class Runner:
    """Compiles the Bass module to a PJRT executable once; reusable calls."""

    def __init__(self, nc, n_cores=NCORES):
        import jax
        import concourse.mybir as _mybir
        from concourse import bass2jax as b2j
        from jax.experimental.shard_map import shard_map
        from jax.sharding import Mesh, PartitionSpec

        b2j.install_neuronx_cc_hook()
        self.nc = nc
        self.n_cores = n_cores
        pname = nc.partition_id_tensor.name if nc.partition_id_tensor else None
        in_names, out_names, out_avals, zero_outs = [], [], [], []
        for alloc in nc.m.functions[0].allocations:
            if not isinstance(alloc, _mybir.MemoryLocationSet):
                continue
            name = alloc.memorylocations[0].name
            if alloc.kind == "ExternalInput":
                if name != pname:
                    in_names.append(name)
            elif alloc.kind == "ExternalOutput":
                shape = tuple(alloc.tensor_shape)
                dtype = _mybir.dt.np(alloc.dtype)
                out_names.append(name)
                out_avals.append(jax.core.ShapedArray(shape, dtype))
                zero_outs.append(np.zeros(shape, dtype))
        self.in_names = list(in_names)
        self.out_names = out_names
        self.out_avals = out_avals
        self.zero_outs = zero_outs
        n_params = len(in_names)
        all_names = in_names + out_names + ([pname] if pname else [])
        donate = tuple(range(n_params, n_params + len(out_names)))
        self.n_params = n_params

        def _body(*args):
            operands = list(args)
            if pname is not None:
                operands.append(b2j.partition_id_tensor())
            outs = b2j._bass_exec_p.bind(
                *operands,
                out_avals=tuple(out_avals),
                in_names=tuple(all_names),
                out_names=tuple(out_names),
                lowering_input_output_aliases=(),
                sim_require_finite=False,
                sim_require_nnan=False,
                nc=nc,
            )
            return tuple(outs)

        devices = jax.devices()[:n_cores]
        mesh = Mesh(np.asarray(devices), ("core",))
        nio = n_params + len(out_names)
        self._jit = jax.jit(
            shard_map(_body, mesh=mesh,
                      in_specs=(PartitionSpec("core"),) * nio,
                      out_specs=(PartitionSpec("core"),) * len(out_names),
                      check_rep=False),
            donate_argnums=donate, keep_unused=True)

    def __call__(self, in_maps, block=True):
        n = self.n_cores
        concat_in = [
            np.concatenate([np.asarray(in_maps[c][name]) for c in range(n)],
                           axis=0)
            for name in self.in_names
        ]
        concat_zero = [
            np.zeros((n * z.shape[0], *z.shape[1:]), z.dtype)
            for z in self.zero_outs
        ]
        out = self._jit(*concat_in, *concat_zero)
        if block:
            for o in out:
                o.block_until_ready()
        return [
            {name: np.asarray(out[i]).reshape(n, *self.out_avals[i].shape)[c]
             for i, name in enumerate(self.out_names)}
            for c in range(n)
        ]


_RUNNER_CACHE = {}


def get_runner(n_cores=NCORES, use_collective=True):
    key = (n_cores, use_collective)
    if key not in _RUNNER_CACHE:
        _RUNNER_CACHE[key] = Runner(_get_nc(n_cores, use_collective), n_cores)
    return _RUNNER_CACHE[key]


def make_in_maps(x, W, b, n_cores=NCORES):
    x = np.asarray(x, dtype=np.float32)
    W = np.asarray(W, dtype=np.float32)
    b = np.asarray(b, dtype=np.float32)
    maps = []
    for c in range(n_cores):
        sl = slice(c * NIS, (c + 1) * NIS)
        maps.append({
            "x": np.ascontiguousarray(x[:, sl, :]),
            "w": np.ascontiguousarray(W[sl]),
            "b": b,
        })
    return maps


def kernel(x, W, b):
    runner = get_runner()
    res = runner(make_in_maps(x, W, b))
    return np.asarray(res[0]["y"], dtype=np.float32)

